# revision 1
# baseline (speedup 1.0000x reference)
"""CostVolume kernel for Trainium2 (8 NeuronCores, batch-sharded).

out[b,h,w,(di,dj)] = mean_c( prv[b,h,w,c] * nxt_pad[b,h+di,w+dj,c] ),  r=4, d=9.

Device strategy (per core, 2 batches):
  - Inputs host-prepped: prv scaled by 1/C, both cast to bf16 and transposed
    to [b, c, h, w] so the channel dim lands on SBUF partitions.
  - TensorEngine: for each (b, h): stationary lhsT = prv row [c, w(128)],
    moving rhs = 9 shifted nxt rows [c, w'] -> PSUM band tile
    [w(128 part), (di 9, w' 128)] f32, contracting c in 2 chunks (128+64).
  - DVE + ACT split the PSUM->SBUF copy (bf16 out).
  - DMA band tiles to DRAM; host gathers the 9 diagonals per (w, di)
    (a skew is not expressible as a Trainium access pattern) into the
    final [B,H,W,81] f32 output during unshard.
"""

import numpy as np
import ml_dtypes

B, H, W, C = 16, 128, 128, 192
R = 4
D = 2 * R + 1  # 9
N_CORES = 8
B_LOC = B // N_CORES  # 2
C0 = 128  # first contraction chunk
C1 = C - C0  # 64
HB = 8  # h rows per DMA block
N_HB = H // HB  # 16

_CACHED = {}


def _build_nc():
    import concourse.mybir as mybir
    from concourse.bacc import Bacc
    from concourse.tile import TileContext

    fp32 = mybir.dt.float32
    bf16 = mybir.dt.bfloat16

    nc = Bacc(
        "TRN2",
        target_bir_lowering=False,
        debug=False,
        num_devices=N_CORES,
    )

    prv_d = nc.dram_tensor("prv_t", [B_LOC, C, H, W], bf16, kind="ExternalInput")
    nxt_d = nc.dram_tensor("nxt_t", [B_LOC, C, H, W], bf16, kind="ExternalInput")
    band_d = nc.dram_tensor("band", [B_LOC, H, W, D, W], bf16, kind="ExternalOutput")

    with TileContext(nc) as tc:
        with (
            tc.tile_pool(name="prv_pool", bufs=2) as prv_pool,
            tc.tile_pool(name="nxt_pool", bufs=4) as nxt_pool,
            tc.tile_pool(name="band_pool", bufs=3) as band_pool,
            tc.tile_pool(name="psum_pool", bufs=2, space="PSUM") as psum_pool,
        ):
            for b in range(B_LOC):
                nxt_tiles = {}  # hb -> (c0_tile, c1_tile)

                def load_nxt(hb):
                    t0 = nxt_pool.tile([C0, HB, W], bf16, tag="nxt_c0")
                    t1 = nxt_pool.tile([C1, HB, W], bf16, tag="nxt_c1")
                    nc.sync.dma_start(t0[:], nxt_d[b, 0:C0, hb * HB:(hb + 1) * HB, :])
                    nc.sync.dma_start(t1[:], nxt_d[b, C0:C, hb * HB:(hb + 1) * HB, :])
                    nxt_tiles[hb] = (t0, t1)

                load_nxt(0)
                load_nxt(1)

                for hb in range(N_HB):
                    if hb + 2 < N_HB and (hb + 2) not in nxt_tiles:
                        load_nxt(hb + 2)

                    p0 = prv_pool.tile([C0, HB, W], bf16, tag="prv_c0")
                    p1 = prv_pool.tile([C1, HB, W], bf16, tag="prv_c1")
                    nc.sync.dma_start(p0[:], prv_d[b, 0:C0, hb * HB:(hb + 1) * HB, :])
                    nc.sync.dma_start(p1[:], prv_d[b, C0:C, hb * HB:(hb + 1) * HB, :])

                    for hl in range(HB):
                        h = hb * HB + hl
                        di_lo = max(0, R - h)
                        di_hi = min(D, H + R - h)

                        psum = psum_pool.tile([W, D, W], fp32, tag="band_ps")
                        for di in range(di_lo, di_hi):
                            h2 = h + di - R
                            for chunk in range(2):
                                lhsT = (p0 if chunk == 0 else p1)[:, hl, :]
                                t = nxt_tiles[h2 // HB][chunk]
                                rhs = t[:, h2 % HB, :]
                                nc.tensor.matmul(
                                    psum[:, di, :], lhsT, rhs,
                                    start=(chunk == 0), stop=(chunk == 1),
                                )

                        band = band_pool.tile([W, D, W], bf16, tag="band_sb")
                        # split the PSUM->SBUF copy between DVE and ACT
                        n_val = di_hi - di_lo
                        di_mid = di_lo + (n_val + 1) // 2
                        nc.vector.tensor_copy(
                            band[:, di_lo:di_mid, :], psum[:, di_lo:di_mid, :]
                        )
                        if di_mid < di_hi:
                            nc.scalar.copy(
                                band[:, di_mid:di_hi, :], psum[:, di_mid:di_hi, :]
                            )
                        nc.sync.dma_start(
                            band_d[b, h, :, di_lo:di_hi, :], band[:, di_lo:di_hi, :]
                        )

    nc.finalize()
    return nc


def _get_nc():
    if "nc" not in _CACHED:
        _CACHED["nc"] = _build_nc()
    return _CACHED["nc"]


def _host_prep(prv, nxt):
    """Scale prv by 1/C, cast to bf16, transpose to [b, c, h, w]."""
    bf16 = ml_dtypes.bfloat16
    prv_t = np.ascontiguousarray(
        (np.asarray(prv, dtype=np.float32) * (1.0 / C)).transpose(0, 3, 1, 2)
    ).astype(bf16)
    nxt_t = np.ascontiguousarray(
        np.asarray(nxt, dtype=np.float32).transpose(0, 3, 1, 2)
    ).astype(bf16)
    return prv_t, nxt_t


def _gather_band(band):
    """band: [B_LOC, H, W, D, W'] (bf16) -> out [B_LOC, H, W, D*D] f32.

    out[b,h,w,di,dj] = band[b,h,w,di, w+dj-R] where the h/w displaced indices
    are in range, else 0.
    """
    band = np.asarray(band, dtype=np.float32)
    padded = np.zeros((B_LOC, H, W, D, W + 2 * R), dtype=np.float32)
    padded[..., R:R + W] = band
    # gather along last axis: idx[w, dj] = w + dj
    idx = (np.arange(W)[:, None] + np.arange(D)[None, :])  # [W, D]
    idx = idx[None, None, :, None, :]  # [1,1,W,1,D]
    idx = np.broadcast_to(idx, (B_LOC, H, W, D, D))
    out = np.take_along_axis(padded, idx, axis=-1)  # [B_LOC, H, W, D(di), D(dj)]
    # zero rows where h + di - R out of range (those band slices are never
    # written on device -> may hold garbage)
    h_idx = np.arange(H)[:, None] + np.arange(D)[None, :] - R  # [H, D]
    h_valid = (h_idx >= 0) & (h_idx < H)  # [H, D]
    out = np.where(h_valid[None, :, None, :, None], out, np.float32(0.0))
    return out.reshape(B_LOC, H, W, D * D)


def kernel(prv, nxt, search_range):
    from concourse.bass_utils import run_bass_kernel_spmd

    assert int(search_range) == R
    prv = np.asarray(prv)
    nxt = np.asarray(nxt)
    assert prv.shape == (B, H, W, C), prv.shape
    out_dtype = prv.dtype if prv.dtype in (np.float32,) else np.float32

    prv_t, nxt_t = _host_prep(prv, nxt)

    in_maps = [
        {
            "prv_t": prv_t[i * B_LOC:(i + 1) * B_LOC],
            "nxt_t": nxt_t[i * B_LOC:(i + 1) * B_LOC],
        }
        for i in range(N_CORES)
    ]

    nc = _get_nc()
    res = run_bass_kernel_spmd(nc, in_maps, list(range(N_CORES)))

    out = np.empty((B, H, W, D * D), dtype=out_dtype)
    for i in range(N_CORES):
        out[i * B_LOC:(i + 1) * B_LOC] = _gather_band(res.results[i]["band"])
    return out



# revision 7
# speedup vs baseline: 3.9079x; 3.9079x over previous
"""CostVolume kernel for Trainium2 (8 NeuronCores, batch-sharded).

out[b,h,w,(di,dj)] = mean_c( prv[b,h,w,c] * nxt_pad[b,h+di,w+dj,c] ),  r=4, d=9.

Device strategy (per core, 2 batches):
  - Host prep: prv scaled by 1/C -> [b, c, H, W] bf16; nxt zero-padded by 4
    -> [b, c, H+8, W+8] bf16 (channel dim on SBUF partitions).
  - Output pixels are tiled into (8 h x 16 w) = 128-pixel blocks. Per block
    and c-chunk (128 + 64), one matmul: stationary lhsT = prv block pixels
    [c, 128], moving rhs = the block's shifted-window of nxt [c, 16 x 24]
    (j = 384 columns) -> PSUM [128 pixels, 384] f32. Every streamed nxt
    column serves up to 81 outputs (vs 9 in a row-band formulation).
  - DVE + ACT alternate draining PSUM -> SBUF (bf16), one DMA per (b, hb)
    writes [128, 8*384] to DRAM.
  - Host gathers the 81 displacement values per pixel from its block window
    (a per-partition diagonal that no lockstep engine can express) and
    returns [B, H, W, 81] f32.
"""

import numpy as np
import ml_dtypes

B, H, W, C = 16, 128, 128, 192
R = 4
D = 2 * R + 1  # 9
N_CORES = 8
B_LOC = B // N_CORES  # 2
C0 = 128  # first contraction chunk
C1 = C - C0  # 64

HL, WS = 8, 16          # block pixel dims (HL*WS = 128 partitions)
HBN, WBN = H // HL, W // WS  # 16, 8
HP, WP = HL + 2 * R, WS + 2 * R  # window dims 16, 24
J = HP * WP             # 384 moving columns per matmul
HPAD, WPAD = H + 2 * R, W + 2 * R  # 136, 136
HHALF = H // 2          # 64 prv rows per half
NXT_HROWS = HHALF + 2 * R  # 72 padded nxt rows per half

_CACHED = {}


def _build_nc():
    import concourse.mybir as mybir
    from concourse.bacc import Bacc
    from concourse.tile import TileContext

    fp32 = mybir.dt.float32
    bf16 = mybir.dt.bfloat16

    nc = Bacc(
        "TRN2",
        target_bir_lowering=False,
        debug=False,
        num_devices=N_CORES,
    )

    # prv pre-tiled on host: last dim = a block's 128 pixels (hl*16+ws),
    # so the matmul stationary AP has a single contiguous free dim.
    prv_d = nc.dram_tensor(
        "prv_t", [B_LOC, C, HBN, WBN, HL * WS], bf16, kind="ExternalInput"
    )
    nxt_d = nc.dram_tensor(
        "nxt_t", [B_LOC, C, HPAD, WPAD], bf16, kind="ExternalInput"
    )
    x_d = nc.dram_tensor(
        "xband", [B_LOC, HBN, HL * WS, WBN * J], bf16, kind="ExternalOutput"
    )

    with TileContext(nc) as tc:
        with (
            tc.tile_pool(name="prv_pool", bufs=2) as prv_pool,
            tc.tile_pool(name="nxt_pool", bufs=2) as nxt_pool,
            tc.tile_pool(name="x_pool", bufs=2) as x_pool,
            tc.tile_pool(name="psum_pool", bufs=4, space="PSUM") as psum_pool,
        ):
            hbh = HBN // 2  # hblocks per half
            for b in range(B_LOC):
                for half in range(2):
                    # input tiles for this half (pool bufs=2 double-buffers
                    # the next half's loads behind this half's compute)
                    p0 = prv_pool.tile([C0, hbh, WBN * HL * WS], bf16, tag="prv_c0")
                    p1 = prv_pool.tile([C1, hbh, WBN * HL * WS], bf16, tag="prv_c1")
                    n0 = nxt_pool.tile([C0, NXT_HROWS, WPAD], bf16, tag="nxt_c0")
                    n1 = nxt_pool.tile([C1, NXT_HROWS, WPAD], bf16, tag="nxt_c1")
                    r0 = half * HHALF
                    hb0 = half * hbh
                    nc.sync.dma_start(p0[:], prv_d[b, 0:C0, hb0:hb0 + hbh, :, :])
                    nc.sync.dma_start(p1[:], prv_d[b, C0:C, hb0:hb0 + hbh, :, :])
                    nc.sync.dma_start(n0[:], nxt_d[b, 0:C0, r0:r0 + NXT_HROWS, :])
                    nc.sync.dma_start(n1[:], nxt_d[b, C0:C, r0:r0 + NXT_HROWS, :])

                    for hb_loc in range(HBN // 2):
                        hb = half * (HBN // 2) + hb_loc
                        xs = x_pool.tile([HL * WS, WBN * J], bf16, tag="xs")
                        for wb in range(WBN):
                            ps = psum_pool.tile([HL * WS, J], fp32, tag="ps")
                            rr = hb_loc * HL
                            cc = wb * WS
                            npix = HL * WS
                            nc.tensor.matmul(
                                ps[:],
                                p0[:, hb_loc, wb * npix:(wb + 1) * npix],
                                n0[:, rr:rr + HP, cc:cc + WP],
                                start=True, stop=False,
                            )
                            nc.tensor.matmul(
                                ps[:],
                                p1[:, hb_loc, wb * npix:(wb + 1) * npix],
                                n1[:, rr:rr + HP, cc:cc + WP],
                                start=False, stop=True,
                            )
                            dst = xs[:, wb * J:(wb + 1) * J]
                            if wb % 2 == 0:
                                nc.vector.tensor_copy(dst, ps[:])
                            else:
                                nc.scalar.copy(dst, ps[:])
                        nc.sync.dma_start(x_d[b, hb, :, :], xs[:])

    nc.finalize()
    return nc


def _get_nc():
    if "nc" not in _CACHED:
        _CACHED["nc"] = _build_nc()
    return _CACHED["nc"]


def _host_prep(prv, nxt):
    """prv: scale by 1/C, block-tiled [b, c, hb, wb, 128] bf16.
    nxt: zero-pad by R, [b, c, h+8, w+8] bf16."""
    bf16 = ml_dtypes.bfloat16
    prv_t = (np.asarray(prv, dtype=np.float32) * (1.0 / C)).transpose(0, 3, 1, 2)
    prv_bt = np.ascontiguousarray(
        prv_t.reshape(B, C, HBN, HL, WBN, WS).transpose(0, 1, 2, 4, 3, 5)
    ).reshape(B, C, HBN, WBN, HL * WS).astype(bf16)
    nxt_t = np.zeros((B, C, HPAD, WPAD), dtype=bf16)
    nxt_t[:, :, R:R + H, R:R + W] = np.asarray(nxt, dtype=np.float32).transpose(
        0, 3, 1, 2
    ).astype(bf16)
    return prv_bt, nxt_t


def _gather_x(x):
    """x: [B_LOC, HBN, 128, WBN*J] bf16 -> out [B_LOC, H, W, 81] f32.

    out[b, hb*8+hl, wb*16+ws, di*9+dj] = x[b, hb, hl*16+ws, wb*384 + (hl+di)*24 + (ws+dj)]
    """
    x = np.asarray(x, dtype=np.float32)
    x7 = x.reshape(B_LOC, HBN, HL, WS, WBN, HP, WP)
    out = np.empty((B_LOC, HBN, HL, WBN, WS, D, D), dtype=np.float32)
    ws_idx = np.arange(WS).reshape(1, 1, WS, 1, 1, 1, 1)
    for hl in range(HL):
        sub = x7[:, :, hl][:, :, :, :, hl:hl + D, :]  # [b, hb, ws, wb, di, WP]
        sw = np.lib.stride_tricks.sliding_window_view(sub, D, axis=5)
        g = np.take_along_axis(sw, ws_idx, axis=5)[:, :, :, :, :, 0, :]
        out[:, :, hl] = g.transpose(0, 1, 3, 2, 4, 5)
    return out.reshape(B_LOC, H, W, D * D)


def kernel(prv, nxt, search_range):
    from concourse.bass_utils import run_bass_kernel_spmd

    assert int(search_range) == R
    prv = np.asarray(prv)
    nxt = np.asarray(nxt)
    assert prv.shape == (B, H, W, C), prv.shape

    prv_t, nxt_t = _host_prep(prv, nxt)

    in_maps = [
        {
            "prv_t": prv_t[i * B_LOC:(i + 1) * B_LOC],
            "nxt_t": nxt_t[i * B_LOC:(i + 1) * B_LOC],
        }
        for i in range(N_CORES)
    ]

    nc = _get_nc()
    res = run_bass_kernel_spmd(nc, in_maps, list(range(N_CORES)))

    out = np.empty((B, H, W, D * D), dtype=np.float32)
    for i in range(N_CORES):
        out[i * B_LOC:(i + 1) * B_LOC] = _gather_x(res.results[i]["xband"])
    return out


# revision 8
# speedup vs baseline: 4.4678x; 1.1433x over previous
"""CostVolume kernel for Trainium2 (8 NeuronCores, batch-sharded).

out[b,h,w,(di,dj)] = mean_c( prv[b,h,w,c] * nxt_pad[b,h+di,w+dj,c] ),  r=4, d=9.

Device strategy (per core, 2 batches):
  - Host prep: prv scaled by 1/C, block-tiled [b, c, hb, wb, 128] bf16 so each
    (8h x 16w)-pixel block is one contiguous matmul-stationary column set;
    nxt zero-padded by 4 -> [b, c, 136, 136] bf16 (c on SBUF partitions).
  - Per 128-pixel block: two PSUM-accumulated matmuls contract c in 96+96
    chunks (both k=96 -> uniform 128x128 PE tile mode, no mode-switch
    drains). Moving rhs = the block's 16x24 shifted window of nxt
    (j=384 columns); stationary = the block's 128 prv pixels. Every
    streamed nxt column serves up to 81 outputs.
  - Inputs stream in h-quarters (bufs=2 double-buffering), DVE + ACT
    alternate draining PSUM->SBUF bf16 two blocks at a time, one DMA per
    (b, hb) writes [128, 8*384] to DRAM.
  - Host gathers the 81 displacement values per pixel from its block window
    (a per-partition diagonal no lockstep engine can express) and returns
    [B, H, W, 81] f32.
"""

import numpy as np
import ml_dtypes

B, H, W, C = 16, 128, 128, 192
R = 4
D = 2 * R + 1  # 9
N_CORES = 8
B_LOC = B // N_CORES  # 2
CK = 96  # contraction chunk (96+96; round_up_size(96)=128 keeps one PE mode)

HL, WS = 8, 16          # block pixel dims (HL*WS = 128 partitions)
HBN, WBN = H // HL, W // WS  # 16, 8
HP, WP = HL + 2 * R, WS + 2 * R  # window dims 16, 24
J = HP * WP             # 384 moving columns per matmul
HPAD, WPAD = H + 2 * R, W + 2 * R  # 136, 136
NQ = 4                  # h-quarters per batch
HBQ = HBN // NQ         # 4 hblocks per quarter
PRV_QROWS = H // NQ     # 32 prv rows per quarter
NXT_QROWS = PRV_QROWS + 2 * R  # 40 padded nxt rows per quarter

_CACHED = {}


def _build_nc():
    import concourse.mybir as mybir
    from concourse.bacc import Bacc
    from concourse.tile import TileContext

    fp32 = mybir.dt.float32
    bf16 = mybir.dt.bfloat16

    nc = Bacc(
        "TRN2",
        target_bir_lowering=False,
        debug=False,
        num_devices=N_CORES,
    )

    prv_d = nc.dram_tensor(
        "prv_t", [B_LOC, C, HBN, WBN, HL * WS], bf16, kind="ExternalInput"
    )
    nxt_d = nc.dram_tensor(
        "nxt_t", [B_LOC, C, HPAD, WPAD], bf16, kind="ExternalInput"
    )
    x_d = nc.dram_tensor(
        "xband", [B_LOC, HBN, HL * WS, WBN * J], bf16, kind="ExternalOutput"
    )

    with TileContext(nc) as tc:
        with (
            tc.tile_pool(name="prv_pool", bufs=2) as prv_pool,
            tc.tile_pool(name="nxt_pool", bufs=2) as nxt_pool,
            tc.tile_pool(name="x_pool", bufs=2) as x_pool,
            tc.tile_pool(name="psum_pool", bufs=3, space="PSUM") as psum_pool,
        ):
            for b in range(B_LOC):
                for q in range(NQ):
                    pa = prv_pool.tile([CK, HBQ, WBN * HL * WS], bf16, tag="prv_a")
                    pb = prv_pool.tile([CK, HBQ, WBN * HL * WS], bf16, tag="prv_b")
                    na = nxt_pool.tile([CK, NXT_QROWS, WPAD], bf16, tag="nxt_a")
                    nb = nxt_pool.tile([CK, NXT_QROWS, WPAD], bf16, tag="nxt_b")
                    r0 = q * PRV_QROWS
                    hb0 = q * HBQ
                    nc.sync.dma_start(pa[:], prv_d[b, 0:CK, hb0:hb0 + HBQ, :, :])
                    nc.sync.dma_start(pb[:], prv_d[b, CK:C, hb0:hb0 + HBQ, :, :])
                    nc.sync.dma_start(na[:], nxt_d[b, 0:CK, r0:r0 + NXT_QROWS, :])
                    nc.sync.dma_start(nb[:], nxt_d[b, CK:C, r0:r0 + NXT_QROWS, :])

                    for hb_loc in range(HBQ):
                        hb = hb0 + hb_loc
                        xs = x_pool.tile([HL * WS, WBN * J], bf16, tag="xs")
                        for wbp in range(WBN // 2):
                            ps = psum_pool.tile([HL * WS, 2, 512], fp32, tag="ps")
                            rr = hb_loc * HL
                            npix = HL * WS
                            for k in range(2):
                                wb = wbp * 2 + k
                                cc = wb * WS
                                nc.tensor.matmul(
                                    ps[:, k, 0:J],
                                    pa[:, hb_loc, wb * npix:(wb + 1) * npix],
                                    na[:, rr:rr + HP, cc:cc + WP],
                                    start=True, stop=False,
                                )
                                nc.tensor.matmul(
                                    ps[:, k, 0:J],
                                    pb[:, hb_loc, wb * npix:(wb + 1) * npix],
                                    nb[:, rr:rr + HP, cc:cc + WP],
                                    start=False, stop=True,
                                )
                            dst = xs[:, wbp * 2 * J:(wbp * 2 + 2) * J]
                            src = ps[:, :, 0:J]
                            if wbp % 2 == 0:
                                nc.vector.tensor_copy(dst, src)
                            else:
                                nc.scalar.copy(dst, src)
                        nc.sync.dma_start(x_d[b, hb, :, :], xs[:])

    nc.finalize()
    return nc


def _get_nc():
    if "nc" not in _CACHED:
        _CACHED["nc"] = _build_nc()
    return _CACHED["nc"]


def _host_prep(prv, nxt):
    """prv: scale by 1/C, block-tiled [b, c, hb, wb, 128] bf16.
    nxt: zero-pad by R, [b, c, h+8, w+8] bf16."""
    bf16 = ml_dtypes.bfloat16
    prv_t = (np.asarray(prv, dtype=np.float32) * (1.0 / C)).transpose(0, 3, 1, 2)
    prv_bt = np.ascontiguousarray(
        prv_t.reshape(B, C, HBN, HL, WBN, WS).transpose(0, 1, 2, 4, 3, 5)
    ).reshape(B, C, HBN, WBN, HL * WS).astype(bf16)
    nxt_t = np.zeros((B, C, HPAD, WPAD), dtype=bf16)
    nxt_t[:, :, R:R + H, R:R + W] = np.asarray(nxt, dtype=np.float32).transpose(
        0, 3, 1, 2
    ).astype(bf16)
    return prv_bt, nxt_t


def _gather_x(x):
    """x: [B_LOC, HBN, 128, WBN*J] bf16 -> out [B_LOC, H, W, 81] f32.

    out[b, hb*8+hl, wb*16+ws, di*9+dj] = x[b, hb, hl*16+ws, wb*384 + (hl+di)*24 + (ws+dj)]
    """
    x = np.asarray(x, dtype=np.float32)
    x7 = x.reshape(B_LOC, HBN, HL, WS, WBN, HP, WP)
    out = np.empty((B_LOC, HBN, HL, WBN, WS, D, D), dtype=np.float32)
    ws_idx = np.arange(WS).reshape(1, 1, WS, 1, 1, 1, 1)
    for hl in range(HL):
        sub = x7[:, :, hl][:, :, :, :, hl:hl + D, :]  # [b, hb, ws, wb, di, WP]
        sw = np.lib.stride_tricks.sliding_window_view(sub, D, axis=5)
        g = np.take_along_axis(sw, ws_idx, axis=5)[:, :, :, :, :, 0, :]
        out[:, :, hl] = g.transpose(0, 1, 3, 2, 4, 5)
    return out.reshape(B_LOC, H, W, D * D)


def kernel(prv, nxt, search_range):
    from concourse.bass_utils import run_bass_kernel_spmd

    assert int(search_range) == R
    prv = np.asarray(prv)
    nxt = np.asarray(nxt)
    assert prv.shape == (B, H, W, C), prv.shape

    prv_t, nxt_t = _host_prep(prv, nxt)

    in_maps = [
        {
            "prv_t": prv_t[i * B_LOC:(i + 1) * B_LOC],
            "nxt_t": nxt_t[i * B_LOC:(i + 1) * B_LOC],
        }
        for i in range(N_CORES)
    ]

    nc = _get_nc()
    res = run_bass_kernel_spmd(nc, in_maps, list(range(N_CORES)))

    out = np.empty((B, H, W, D * D), dtype=np.float32)
    for i in range(N_CORES):
        out[i * B_LOC:(i + 1) * B_LOC] = _gather_x(res.results[i]["xband"])
    return out


# revision 11
# speedup vs baseline: 5.2223x; 1.1689x over previous
"""CostVolume kernel for Trainium2 (8 NeuronCores, batch-sharded).

out[b,h,w,(di,dj)] = mean_c( prv[b,h,w,c] * nxt_pad[b,h+di,w+dj,c] ),  r=4, d=9.

Device strategy (per core, 2 batches):
  - Host prep: prv scaled by 1/C, block-tiled [b, c, hb, wb, 128] bf16 so each
    (8h x 16w)-pixel block is one contiguous matmul-stationary column set;
    nxt zero-padded by 4 -> [b, c, 136, 136] bf16 (c on SBUF partitions).
  - Per 128-pixel block: two PSUM-accumulated matmuls contract c in 96+96
    chunks (both k=96 -> uniform 128x128 PE tile mode, no mode-switch
    drains). Moving rhs = the block's 16x24 shifted window of nxt
    (j=384 columns); stationary = the block's 128 prv pixels. Every
    streamed nxt column serves up to 81 outputs.
  - Inputs stream in h-quarters (bufs=2 double-buffering), DVE + ACT
    alternate draining PSUM->SBUF bf16 two blocks at a time, one DMA per
    (b, hb) writes [128, 8*384] to DRAM.
  - Host gathers the 81 displacement values per pixel from its block window
    (a per-partition diagonal no lockstep engine can express) and returns
    [B, H, W, 81] f32.
"""

import numpy as np
import ml_dtypes

B, H, W, C = 16, 128, 128, 192
R = 4
D = 2 * R + 1  # 9
N_CORES = 8
B_LOC = B // N_CORES  # 2
CK = 96  # contraction chunk (96+96; round_up_size(96)=128 keeps one PE mode)

HL, WS = 8, 16          # block pixel dims (HL*WS = 128 partitions)
HBN, WBN = H // HL, W // WS  # 16, 8
HP, WP = HL + 2 * R, WS + 2 * R  # window dims 16, 24
J = HP * WP             # 384 moving columns per matmul
HPAD, WPAD = H + 2 * R, W + 2 * R  # 136, 136
NQ = 4                  # h-quarters per batch
HBQ = HBN // NQ         # 4 hblocks per quarter
PRV_QROWS = H // NQ     # 32 prv rows per quarter
NXT_QROWS = PRV_QROWS + 2 * R  # 40 padded nxt rows per quarter

_CACHED = {}


def _build_nc():
    import concourse.mybir as mybir
    from concourse.bacc import Bacc
    from concourse.tile import TileContext

    fp32 = mybir.dt.float32
    bf16 = mybir.dt.bfloat16

    nc = Bacc(
        "TRN2",
        target_bir_lowering=False,
        debug=False,
        num_devices=N_CORES,
    )

    prv_d = nc.dram_tensor(
        "prv_t", [B_LOC, C, HBN, WBN, HL * WS], bf16, kind="ExternalInput"
    )
    nxt_d = nc.dram_tensor(
        "nxt_t", [B_LOC, C, HPAD, WPAD], bf16, kind="ExternalInput"
    )
    x_d = nc.dram_tensor(
        "xband", [B_LOC, HBN, HL * WS, WBN * J], bf16, kind="ExternalOutput"
    )

    with TileContext(nc) as tc:
        with (
            tc.tile_pool(name="prv_pool", bufs=2) as prv_pool,
            tc.tile_pool(name="nxt_pool", bufs=2) as nxt_pool,
            tc.tile_pool(name="x_pool", bufs=3) as x_pool,
            tc.tile_pool(name="psum_pool", bufs=4, space="PSUM") as psum_pool,
        ):
            for b in range(B_LOC):
                for q in range(NQ):
                    pa = prv_pool.tile([CK, HBQ, WBN * HL * WS], bf16, tag="prv_a")
                    pb = prv_pool.tile([CK, HBQ, WBN * HL * WS], bf16, tag="prv_b")
                    na = nxt_pool.tile([CK, NXT_QROWS, WPAD], bf16, tag="nxt_a")
                    nb = nxt_pool.tile([CK, NXT_QROWS, WPAD], bf16, tag="nxt_b")
                    r0 = q * PRV_QROWS
                    hb0 = q * HBQ
                    # chunk-A tiles first: the first matmul of the quarter
                    # only needs (pa, na), so compute starts sooner
                    nc.sync.dma_start(pa[:], prv_d[b, 0:CK, hb0:hb0 + HBQ, :, :])
                    nc.sync.dma_start(na[:], nxt_d[b, 0:CK, r0:r0 + NXT_QROWS, :])
                    nc.sync.dma_start(pb[:], prv_d[b, CK:C, hb0:hb0 + HBQ, :, :])
                    nc.sync.dma_start(nb[:], nxt_d[b, CK:C, r0:r0 + NXT_QROWS, :])

                    for hb_loc in range(HBQ):
                        hb = hb0 + hb_loc
                        xs = x_pool.tile([HL * WS, WBN * J], bf16, tag="xs")
                        for wbp in range(WBN // 2):
                            ps = psum_pool.tile([HL * WS, 2, 512], fp32, tag="ps")
                            rr = hb_loc * HL
                            npix = HL * WS
                            for k in range(2):
                                wb = wbp * 2 + k
                                cc = wb * WS
                                nc.tensor.matmul(
                                    ps[:, k, 0:J],
                                    pa[:, hb_loc, wb * npix:(wb + 1) * npix],
                                    na[:, rr:rr + HP, cc:cc + WP],
                                    start=True, stop=False,
                                )
                                nc.tensor.matmul(
                                    ps[:, k, 0:J],
                                    pb[:, hb_loc, wb * npix:(wb + 1) * npix],
                                    nb[:, rr:rr + HP, cc:cc + WP],
                                    start=False, stop=True,
                                )
                            dst = xs[:, wbp * 2 * J:(wbp * 2 + 2) * J]
                            src = ps[:, :, 0:J]
                            if wbp % 2 == 0:
                                nc.vector.tensor_copy(dst, src)
                            else:
                                nc.scalar.copy(dst, src)
                        # stores ride the ACT HWDGE ring so they interleave
                        # with input loads on the sync ring
                        nc.scalar.dma_start(x_d[b, hb, :, :], xs[:])

    nc.finalize()
    return nc


def _get_nc():
    if "nc" not in _CACHED:
        _CACHED["nc"] = _build_nc()
    return _CACHED["nc"]


def _host_prep(prv, nxt):
    """prv: scale by 1/C, block-tiled [b, c, hb, wb, 128] bf16.
    nxt: zero-pad by R, [b, c, h+8, w+8] bf16."""
    bf16 = ml_dtypes.bfloat16
    prv_t = (np.asarray(prv, dtype=np.float32) * (1.0 / C)).transpose(0, 3, 1, 2)
    prv_bt = np.ascontiguousarray(
        prv_t.reshape(B, C, HBN, HL, WBN, WS).transpose(0, 1, 2, 4, 3, 5)
    ).reshape(B, C, HBN, WBN, HL * WS).astype(bf16)
    nxt_t = np.zeros((B, C, HPAD, WPAD), dtype=bf16)
    nxt_t[:, :, R:R + H, R:R + W] = np.asarray(nxt, dtype=np.float32).transpose(
        0, 3, 1, 2
    ).astype(bf16)
    return prv_bt, nxt_t


def _gather_x(x):
    """x: [B_LOC, HBN, 128, WBN*J] bf16 -> out [B_LOC, H, W, 81] f32.

    out[b, hb*8+hl, wb*16+ws, di*9+dj] = x[b, hb, hl*16+ws, wb*384 + (hl+di)*24 + (ws+dj)]
    """
    x = np.asarray(x, dtype=np.float32)
    x7 = x.reshape(B_LOC, HBN, HL, WS, WBN, HP, WP)
    out = np.empty((B_LOC, HBN, HL, WBN, WS, D, D), dtype=np.float32)
    ws_idx = np.arange(WS).reshape(1, 1, WS, 1, 1, 1, 1)
    for hl in range(HL):
        sub = x7[:, :, hl][:, :, :, :, hl:hl + D, :]  # [b, hb, ws, wb, di, WP]
        sw = np.lib.stride_tricks.sliding_window_view(sub, D, axis=5)
        g = np.take_along_axis(sw, ws_idx, axis=5)[:, :, :, :, :, 0, :]
        out[:, :, hl] = g.transpose(0, 1, 3, 2, 4, 5)
    return out.reshape(B_LOC, H, W, D * D)


def kernel(prv, nxt, search_range):
    from concourse.bass_utils import run_bass_kernel_spmd

    assert int(search_range) == R
    prv = np.asarray(prv)
    nxt = np.asarray(nxt)
    assert prv.shape == (B, H, W, C), prv.shape

    prv_t, nxt_t = _host_prep(prv, nxt)

    in_maps = [
        {
            "prv_t": prv_t[i * B_LOC:(i + 1) * B_LOC],
            "nxt_t": nxt_t[i * B_LOC:(i + 1) * B_LOC],
        }
        for i in range(N_CORES)
    ]

    nc = _get_nc()
    res = run_bass_kernel_spmd(nc, in_maps, list(range(N_CORES)))

    out = np.empty((B, H, W, D * D), dtype=np.float32)
    for i in range(N_CORES):
        out[i * B_LOC:(i + 1) * B_LOC] = _gather_x(res.results[i]["xband"])
    return out
